# revision 6
# baseline (speedup 1.0000x reference)
"""Trainium2 Bass kernel for nn_NeuralGPKernel (sparse_attention).

Self-contained: hardcodes all shapes. Shards (B=2) x (N_q in 4 chunks of 128)
across 8 NeuronCores; each core computes mean/var for its 128 queries.

Math restructuring vs the reference:
  - ki @ kw1 decomposes: ki = [pos_q, pos_o, pos_q - pos_o], so
    hidden[q,o,:] = u[q,:] + w[o,:] with u = pos_q @ (A+C), w = pos_o @ (B-C) + kb1
    where kw1 = [A; B; C] (3 blocks of 3 rows).
  - softmax row-constants dropped: kb2 and |pos_q|^2 terms cancel in softmax.
  - log(rbf + 1e-8) ~= -dist2 / (sigma^2 + 1e-6)  (error <= 2e-7 for dist2<=3).
  - weighted variance = E[v^2] - E[v]^2 (weights sum to 1).

Pipeline design (v2): the kernel is relu-production bound: 256 tiles of
relu(w[o,k]+u[q,k]) [128,512] split DVE (~348ns) / ACT (~706ns).  Everything
else is arranged to keep those two engines saturated:
  - h_obs pre-transposed on host -> no feature-net transposes on device.
  - W^T for attention via dma_start_transpose (sync queue + DMA engines).
  - feature net runs at startup; attention split blocks 0-5 / 6-7.
  - per-block exp/normalize emitted at lagged positions in the engine streams.
"""

import os
import sys
import types
import numpy as np

B, N_O, N_Q = 2, 512, 512
POS_DIM, LATENT, HEADS, HEAD_DIM, OUT_DIM = 3, 256, 8, 32, 128
HD = HEADS * HEAD_DIM
N_CORES = 8
NQ_C = N_Q * B // N_CORES  # 128 queries per core
QB = 16                     # queries per logits block
NBLK = NQ_C // QB           # 8 blocks per core

LAST_RESULT = None  # test.py reads exec_time_ns from here

N_A_TILES = int(os.environ.get("KERNEL_NA", "79"))  # relu tiles given to ACT
EXP_LAG_POS = int(os.environ.get("KERNEL_EXPLAG", "12"))   # tile idx in next blk
NORM_LAG_POS = int(os.environ.get("KERNEL_NORMLAG", "20"))


def _relu_pattern():
    """256-char D/A assignment, A spread evenly (N_A_TILES of them)."""
    pat = []
    acc = 0
    for t in range(256):
        acc += N_A_TILES
        if acc >= 256:
            acc -= 256
            pat.append("A")
        else:
            pat.append("D")
    return "".join(pat)


def _install_ntff_hook():
    if "antenv.axon_hooks" in sys.modules:
        return
    try:
        import trn_agent_boot.trn_boot as tb
        hook = tb._ntff_profile_via_ctypes("/opt/axon/libaxon_pjrt.so")
    except Exception:
        hook = None
    m = types.ModuleType("antenv.axon_hooks")
    m.get_axon_ntff_profile_hook = lambda: hook
    m.set_axon_ntff_profile_hook = lambda h: None
    sys.modules["antenv.axon_hooks"] = m


def build_program():
    import concourse.bass as bass
    import concourse.mybir as mybir
    import concourse.tile as tile
    from concourse import bacc
    from contextlib import ExitStack

    f32 = mybir.dt.float32
    bf16 = mybir.dt.bfloat16
    ALU = mybir.AluOpType
    AF = mybir.ActivationFunctionType

    nc = bacc.Bacc("TRN2", target_bir_lowering=False, debug=False)

    def din(name, shape):
        return nc.dram_tensor(name, shape, f32, kind="ExternalInput").ap()

    def dout(name, shape):
        return nc.dram_tensor(name, shape, f32, kind="ExternalOutput").ap()

    h_obs_T = din("h_obs_T", [LATENT, N_O])
    pos_obs_T = din("pos_obs_T", [POS_DIM, N_O])
    pos_q_T = din("pos_q_T", [POS_DIM, NQ_C])
    fw1 = din("fw1", [LATENT, LATENT])
    fb1 = din("fb1", [LATENT])
    fw2 = din("fw2", [LATENT, HD])
    fb2 = din("fb2", [HD])
    log_sigma = din("log_sigma", [HEADS])
    kw1 = din("kw1", [POS_DIM * 3, LATENT])
    kb1 = din("kb1", [LATENT])
    kw2 = din("kw2", [LATENT, HEADS])
    ow = din("ow", [HD, OUT_DIM])
    ob = din("ob", [OUT_DIM])
    vw = din("vw", [HD, OUT_DIM])
    vb = din("vb", [OUT_DIM])
    mean_o = dout("mean", [NQ_C, OUT_DIM])
    var_o = dout("var", [NQ_C, OUT_DIM])

    def ap(t, offset, dims):
        return bass.AP(tensor=t.tensor, offset=t.offset + offset, ap=list(dims))

    PAT = _relu_pattern()

    with tile.TileContext(nc) as tc:
        st = ExitStack()
        _keep = []

        def T(shape, name, dt=f32):
            t, free = tc.tile(shape, dt, name=name)
            _keep.append(free)
            return t

        # ---------------- persistent SBUF tiles ----------------
        ls_rep = T([128, 1], "ls_rep")        # log_sigma[h] at partition (q*8+h)
        s2inv_neg = T([128, 1], "s2inv_neg")  # -1/(exp(2 ls)+1e-6)
        s2rep4 = T([4, 128], "s2rep4")
        w_bd = [T([128, 128], f"w_bd{k}", bf16) for k in range(2)]
        kw2_2 = T([128, 2 * HEADS], "kw2_2")
        k3 = T([3, 3 * LATENT], "k3")         # kw1 blocks A|B|C side by side
        AC = T([3, LATENT], "AC")
        BC = T([3, LATENT], "BC")
        lhsT_df = T([3, NQ_C], "lhsT_df")
        pq4 = T([4, NQ_C], "pq4")
        pos_oT = T([3, N_O], "pos_oT")
        rhs_d = T([4, N_O], "rhs_d", bf16)
        sq3 = T([3, N_O], "sq3")
        ones3 = T([3, 1], "ones3")
        ones1 = T([1, 128], "ones1")
        tmp_po2 = T([1, N_O], "tmp_po2", bf16)
        dh_all = T([4, NBLK * 128], "dh_all", bf16)
        u_kt = [T([128, NQ_C], f"u_kt{k}") for k in range(2)]
        wT = [T([128, N_O], f"wT{k}", bf16) for k in range(2)]
        kb1_2 = T([128, 2], "kb1_2")
        fb1_2 = T([128, 2], "fb1_2")
        fb2_row = T([1, HD], "fb2_row")
        ob_row = T([1, OUT_DIM], "ob_row")
        vb_row = T([1, OUT_DIM], "vb_row")
        fw1_all = T([128, 2 * LATENT], "fw1_all")
        fw2_all = T([128, 2 * HD], "fw2_all")
        ow_all = T([128, 2 * OUT_DIM], "ow_all")
        vw_all = T([128, 2 * OUT_DIM], "vw_all")
        hT_all = T([128, 2 * N_O], "hT_all")
        hidT = [T([128, N_O], f"hidT{k}") for k in range(2)]
        v_sb = [T([128, HD], f"v{k}", bf16) for k in range(4)]
        v2_sb = [T([128, HD], f"v2{k}", bf16) for k in range(4)]
        WT_all = T([128, 4 * NQ_C * HEADS], "WT_all", bf16)  # [o, ot, i, (j,h)]
        sums_all = T([128, NBLK], "sums_all")
        recip_b = [T([128, 1], f"recip{i}") for i in range(NBLK)]
        hqT = [T([128, NQ_C], f"hqT{k}") for k in range(2)]
        sqT = [T([128, NQ_C], f"sqT{k}") for k in range(2)]
        varT = [T([128, NQ_C], f"varT{k}") for k in range(2)]
        mean_sb = T([NQ_C, OUT_DIM], "mean_sb")
        var_sb = T([NQ_C, OUT_DIM], "var_sb")

        # ---------------- DMA issue (spread across queues) ----------------
        # scalar queue: pos_q_T (feeds psu -> u_kt quickly)
        nc.scalar.dma_start(out=lhsT_df[:], in_=pos_q_T[:])
        # sync queue: pos_obs_T, kw1 blocks, kw2, kb1
        nc.sync.dma_start(out=pos_oT[:], in_=pos_obs_T[:])
        nc.sync.dma_start(
            out=k3[:], in_=ap(kw1, 0, [[LATENT, 3], [3 * LATENT, 3], [1, LATENT]]))
        nc.sync.dma_start(
            out=kw2_2[:],
            in_=ap(kw2, 0, [[HEADS, 128], [128 * HEADS, 2], [1, HEADS]]))
        nc.sync.dma_start(
            out=kb1_2[:], in_=ap(kb1, 0, [[1, 128], [128, 2]]))
        # gpsimd queue: everything else, ordered by need-time
        nc.gpsimd.dma_start(out=ls_rep[:], in_=ap(log_sigma, 0, [[0, 16], [1, 8]]))
        nc.gpsimd.dma_start(out=pq4[0:3, :], in_=pos_q_T[:])
        nc.gpsimd.dma_start(
            out=hT_all[:],
            in_=ap(h_obs_T, 0, [[N_O, 128], [128 * N_O, 2], [1, N_O]]))
        nc.gpsimd.dma_start(
            out=fw1_all[:],
            in_=ap(fw1, 0, [[LATENT, 128], [128 * LATENT, 2], [1, LATENT]]))
        nc.gpsimd.dma_start(out=fb1_2[:], in_=ap(fb1, 0, [[1, 128], [128, 2]]))

        # ---------------- startup compute ----------------
        # s2 chain: exp on ACT, rest tiny DVE ops
        nc.scalar.activation(out=s2inv_neg[:], in_=ls_rep[:], func=AF.Exp, scale=2.0)
        nc.vector.memset(ones3[:], 1.0)
        nc.vector.memset(ones1[:], 1.0)
        nc.vector.tensor_scalar_add(out=s2inv_neg[:], in0=s2inv_neg[:], scalar1=1e-6)
        nc.vector.reciprocal(out=s2inv_neg[:], in_=s2inv_neg[:])
        nc.vector.tensor_add(AC[:], k3[:, 0:LATENT], k3[:, 2 * LATENT:3 * LATENT])
        nc.vector.tensor_sub(BC[:], k3[:, LATENT:2 * LATENT], k3[:, 2 * LATENT:])
        nc.scalar.mul(out=s2inv_neg[:], in_=s2inv_neg[:], mul=-1.0)
        nc.scalar.mul(out=rhs_d[0:3, :], in_=pos_oT[:], mul=-2.0)
        nc.vector.tensor_mul(sq3[:], pos_oT[:], pos_oT[:])

        # remaining gpsimd DMAs (s2rep4 needs s2inv_neg computed above)
        nc.gpsimd.dma_start(out=pq4[3:4, :], in_=ones1[0:1, :])
        for r in range(4):
            nc.gpsimd.dma_start(out=s2rep4[r:r + 1, :], in_=s2inv_neg[:, 0:1])
        nc.gpsimd.dma_start(
            out=fw2_all[:],
            in_=ap(fw2, 0, [[HD, 128], [128 * HD, 2], [1, HD]]))
        nc.gpsimd.dma_start(out=fb2_row[:], in_=fb2[:])
        nc.gpsimd.dma_start(
            out=ow_all[:],
            in_=ap(ow, 0, [[OUT_DIM, 128], [128 * OUT_DIM, 2], [1, OUT_DIM]]))
        nc.gpsimd.dma_start(
            out=vw_all[:],
            in_=ap(vw, 0, [[OUT_DIM, 128], [128 * OUT_DIM, 2], [1, OUT_DIM]]))
        nc.gpsimd.dma_start(out=ob_row[:], in_=ob[:])
        nc.gpsimd.dma_start(out=vb_row[:], in_=vb[:])

        st0 = st.enter_context(ExitStack())
        pp_pre = st0.enter_context(tc.tile_pool(name="pp_pre", bufs=2, space="PSUM"))
        pp_feat = st0.enter_context(tc.tile_pool(name="pp_feat", bufs=2, space="PSUM"))

        # u^T and w^T (kernel-MLP rank factors)
        for mt in range(2):
            psw = pp_pre.tile([128, N_O], f32, tag="pre", name="psw")
            nc.tensor.matmul(
                psw[:], lhsT=BC[:, 128 * mt:128 * (mt + 1)], rhs=pos_oT[:],
                start=True, stop=True)
            nc.scalar.activation(
                out=wT[mt][:], in_=psw[:], func=AF.Identity,
                bias=kb1_2[:, mt:mt + 1])
            psu = pp_pre.tile([128, NQ_C], f32, tag="pre", name="psu")
            nc.tensor.matmul(
                psu[:], lhsT=AC[:, 128 * mt:128 * (mt + 1)], rhs=lhsT_df[:],
                start=True, stop=True)
            nc.vector.tensor_copy(out=u_kt[mt][:], in_=psu[:])

        # |pos_o|^2 row of rhs_d
        ps1 = pp_pre.tile([1, N_O], f32, tag="po2", name="ps1")
        nc.tensor.matmul(ps1[:], lhsT=ones3[:], rhs=sq3[:], start=True, stop=True)
        nc.vector.tensor_copy(out=tmp_po2[:], in_=ps1[:])
        nc.gpsimd.dma_start(out=rhs_d[3:4, :], in_=tmp_po2[:])

        # block-diagonal kw2 (col-tiled): variant jj at cols [40jj, 40jj+8)
        for k in range(2):
            nc.vector.memset(w_bd[k][:], 0.0)
            for jj in range(4):
                nc.vector.tensor_copy(
                    out=w_bd[k][:, 40 * jj:40 * jj + 8],
                    in_=kw2_2[:, 8 * k:8 * (k + 1)])

        # all dist2 lhsT tiles (dh): col (8j+h): rows 0-2 pos_q*(-1/s2), row3 -1/s2
        for i in range(NBLK):
            _o = dh_all[:, 128 * i:128 * (i + 1)]
            _p = pq4[:]
            _s = s2rep4[:]
            nc.vector.tensor_tensor(
                out=ap(_o, 0, [_o.ap[0], [8, QB], [1, 8]]),
                in0=ap(_p, QB * i, [_p.ap[0], [1, QB], [0, 8]]),
                in1=ap(_s, 0, [_s.ap[0], [8, QB], [1, 8]]),
                op=ALU.mult)

        # feature net: hidden = relu(h fw1 + fb1); v = hidden fw2 + fb2
        for mt in range(2):
            psh = pp_feat.tile([128, N_O], f32, tag="f", name="psh")
            for kt in range(2):
                nc.tensor.matmul(
                    psh[:],
                    lhsT=fw1_all[:, 256 * kt + 128 * mt:256 * kt + 128 * (mt + 1)],
                    rhs=hT_all[:, 512 * kt:512 * (kt + 1)],
                    start=(kt == 0), stop=(kt == 1))
            nc.scalar.activation(
                out=hidT[mt][:], in_=psh[:], func=AF.Relu, bias=fb1_2[:, mt:mt + 1])
        for ot in range(4):
            psv = pp_feat.tile([128, HD], f32, tag="f", name="psv")
            for mt in range(2):
                nc.tensor.matmul(
                    psv[:], lhsT=hidT[mt][:, 128 * ot:128 * (ot + 1)],
                    rhs=fw2_all[:, 256 * mt:256 * (mt + 1)],
                    start=(mt == 0), stop=False)
            nc.tensor.matmul(psv[:], lhsT=ones1[:], rhs=fb2_row[:],
                             start=False, stop=True)
            nc.scalar.copy(out=v_sb[ot][:], in_=psv[:])
            nc.vector.tensor_mul(v2_sb[ot][:], v_sb[ot][:], v_sb[ot][:])

        # ---------------- block loop ----------------
        st0.close()
        spool = st.enter_context(tc.tile_pool(name="spool", bufs=16))
        wpool = st.enter_context(tc.tile_pool(name="wpool", bufs=3))
        pp_l = st.enter_context(tc.tile_pool(name="pp_l", bufs=4, space="PSUM"))
        pp_att = st.enter_context(tc.tile_pool(name="pp_att", bufs=1, space="PSUM"))

        pm_all = pp_att.tile([128, 2 * NQ_C], f32, tag="pm", name="pm_all")
        pe_all = pp_att.tile([128, 2 * NQ_C], f32, tag="pe", name="pe_all")
        pm = [pm_all[:, NQ_C * k:NQ_C * (k + 1)] for k in range(2)]
        pe = [pe_all[:, NQ_C * k:NQ_C * (k + 1)] for k in range(2)]

        W_ts = [None] * NBLK
        lps_all = [None] * NBLK

        def emit_post(i):
            """exp + recip + normalize + dma-transpose for block i."""
            lps = lps_all[i]
            W_t = wpool.tile([128, N_O], bf16, tag="W", name="W_t")
            W_ts[i] = W_t
            nc.scalar.activation(
                out=W_t[:], in_=lps[:], func=AF.Exp,
                accum_out=sums_all[:, i:i + 1])
            nc.vector.reciprocal(out=recip_b[i][:], in_=sums_all[:, i:i + 1])
            nc.vector.tensor_scalar_mul(
                out=W_t[:], in0=W_t[:], scalar1=recip_b[i][:, 0:1])
            _w = WT_all[:]
            out3d = bass.AP(
                tensor=_w.tensor, offset=_w.offset + 128 * i,
                ap=[_w.ap[0], [NQ_C * HEADS, 4], [1, 128]])
            nc.sync.dma_start_transpose(out=out3d, in_=W_t[:])

        def emit_block(i):
            lps = pp_l.tile([128, N_O], f32, tag="logits", name="lps")
            lps_all[i] = lps
            tidx = 0
            for kt in range(2):
                for wave in range(4):
                    stiles = []
                    for g in range(4):
                        j = 4 * g + wave
                        qg = QB * i + j
                        s_t = spool.tile([128, N_O], bf16, tag="s", name="s_t")
                        ucol = u_kt[kt][:, qg:qg + 1]
                        eng = PAT[32 * i + tidx]
                        if eng == "D":
                            nc.vector.tensor_scalar(
                                out=s_t[:], in0=wT[kt][:], scalar1=ucol,
                                scalar2=0.0, op0=ALU.add, op1=ALU.max)
                        else:
                            nc.scalar.activation(
                                out=s_t[:], in_=wT[kt][:], func=AF.Relu,
                                bias=ucol)
                        stiles.append((g, s_t))
                        tidx += 1
                        if i >= 1 and tidx == EXP_LAG_POS:
                            emit_post(i - 1)
                    for g, s_t in stiles:
                        nc.tensor.matmul(
                            lps[32 * g:32 * g + 32, :],
                            lhsT=w_bd[kt][:, 32 * wave:32 * (wave + 1)],
                            rhs=s_t[:],
                            start=(kt == 0 and wave == 0), stop=False,
                            tile_position=(0, 32 * g), skip_group_check=True)
            nc.tensor.matmul(
                lps[:], lhsT=dh_all[:, 128 * i:128 * (i + 1)], rhs=rhs_d[:],
                start=False, stop=True, skip_group_check=True)

        def emit_attention(i0, i1):
            """accumulate pm/pe for blocks [i0, i1)."""
            nb = i1 - i0
            c0 = QB * i0
            for h in range(HEADS):
                k = h // 4
                r0 = 32 * (h % 4)
                for ot in range(4):
                    rhs = ap(WT_all[:], 1024 * ot + 128 * i0 + h,
                             [WT_all[:].ap[0], [128, nb], [8, QB]])
                    nc.tensor.matmul(
                        pm[k][r0:r0 + 32, c0:c0 + QB * nb],
                        lhsT=v_sb[ot][:, 32 * h:32 * (h + 1)], rhs=rhs,
                        start=(ot == 0), stop=(ot == 3), tile_position=(0, r0))
                    nc.tensor.matmul(
                        pe[k][r0:r0 + 32, c0:c0 + QB * nb],
                        lhsT=v2_sb[ot][:, 32 * h:32 * (h + 1)], rhs=rhs,
                        start=(ot == 0), stop=(ot == 3), tile_position=(0, r0))

        for i in range(NBLK):
            emit_block(i)
            if i == NBLK - 1:
                emit_attention(0, 6)
        emit_post(NBLK - 1)
        emit_attention(6, 8)

        # ---------------- tail: mean/var ----------------
        pp_t = st.enter_context(tc.tile_pool(name="pp_t", bufs=2, space="PSUM"))
        for k in range(2):
            nc.vector.tensor_copy(out=hqT[k][:], in_=pm[k][:])
            nc.vector.tensor_mul(sqT[k][:], hqT[k][:], hqT[k][:])
            nc.vector.tensor_sub(varT[k][:], pe[k][:], sqT[k][:])

        pso = pp_t.tile([NQ_C, OUT_DIM], f32, tag="t", name="pso")
        for k in range(2):
            nc.tensor.matmul(pso[:], lhsT=hqT[k][:], rhs=ow_all[:, 128 * k:128 * (k + 1)],
                             start=(k == 0), stop=False)
        nc.tensor.matmul(pso[:], lhsT=ones1[:], rhs=ob_row[:], start=False, stop=True)
        nc.vector.tensor_copy(out=mean_sb[:], in_=pso[:])
        nc.sync.dma_start(out=mean_o[:], in_=mean_sb[:])

        psv2 = pp_t.tile([NQ_C, OUT_DIM], f32, tag="t", name="psv2")
        for k in range(2):
            nc.tensor.matmul(psv2[:], lhsT=varT[k][:], rhs=vw_all[:, 128 * k:128 * (k + 1)],
                             start=(k == 0), stop=False)
        nc.tensor.matmul(psv2[:], lhsT=ones1[:], rhs=vb_row[:], start=False, stop=True)
        # softplus(x) = ln(1 + exp(x))
        nc.scalar.activation(out=var_sb[:], in_=psv2[:], func=AF.Exp)
        nc.vector.tensor_scalar_add(out=var_sb[:], in0=var_sb[:], scalar1=1.0)
        nc.scalar.activation(out=var_sb[:], in_=var_sb[:], func=AF.Ln)
        nc.sync.dma_start(out=var_o[:], in_=var_sb[:])

        st.close()
        for f in reversed(_keep):
            f()

    nc.compile()
    return nc


_NC = None


def _get_nc():
    global _NC
    if _NC is None:
        _NC = build_program()
    return _NC


def shard_inputs(inputs):
    """Build per-core input maps from full inputs."""
    g = {k: np.ascontiguousarray(np.asarray(v, dtype=np.float32))
         for k, v in inputs.items()}
    hT = [np.ascontiguousarray(g["h_obs"][b].T) for b in range(B)]
    poT = [np.ascontiguousarray(g["pos_obs"][b].T) for b in range(B)]
    maps = []
    for c in range(N_CORES):
        b, qi = c // 4, c % 4
        maps.append({
            "h_obs_T": hT[b],
            "pos_obs_T": poT[b],
            "pos_q_T": np.ascontiguousarray(
                g["pos_query"][b, 128 * qi:128 * (qi + 1)].T),
            "fw1": g["fw1"], "fb1": g["fb1"], "fw2": g["fw2"], "fb2": g["fb2"],
            "log_sigma": g["log_sigma"],
            "kw1": g["kw1"], "kb1": g["kb1"], "kw2": g["kw2"],
            "ow": g["ow"], "ob": g["ob"], "vw": g["vw"], "vb": g["vb"],
        })
    return maps


def kernel(**inputs):
    global LAST_RESULT
    _install_ntff_hook()
    from concourse.bass_utils import run_bass_kernel_spmd

    nc = _get_nc()
    maps = shard_inputs(inputs)
    trace = bool(int(os.environ.get("KERNEL_TRACE", "0")))
    res = run_bass_kernel_spmd(nc, maps, list(range(N_CORES)), trace=trace)
    LAST_RESULT = res
    mean = np.zeros((B, N_Q, OUT_DIM), np.float32)
    var = np.zeros((B, N_Q, OUT_DIM), np.float32)
    for c in range(N_CORES):
        b, qi = c // 4, c % 4
        mean[b, 128 * qi:128 * (qi + 1)] = res.results[c]["mean"]
        var[b, 128 * qi:128 * (qi + 1)] = res.results[c]["var"]
    return (mean, var)


# revision 10
# speedup vs baseline: 1.3611x; 1.3611x over previous
"""Trainium2 Bass kernel for nn_NeuralGPKernel (sparse_attention).

Self-contained: hardcodes all shapes. Shards (B=2) x (N_q in 4 chunks of 128)
across 8 NeuronCores; each core computes mean/var for its 128 queries.

Math restructuring vs the reference:
  - ki @ kw1 decomposes: ki = [pos_q, pos_o, pos_q - pos_o], so
    hidden[q,o,:] = u[q,:] + w[o,:] with u = pos_q @ (A+C), w = pos_o @ (B-C) + kb1
    where kw1 = [A; B; C] (3 blocks of 3 rows).
  - softmax row-constants dropped: kb2 and |pos_q|^2 terms cancel in softmax.
  - log(rbf + 1e-8) ~= -dist2 / (sigma^2 + 1e-6)  (error <= 2e-7 for dist2<=3).
  - weighted variance = E[v^2] - E[v]^2 (weights sum to 1).

Pipeline design (v2): the kernel is relu-production bound: 256 tiles of
relu(w[o,k]+u[q,k]) [128,512] split DVE (~348ns) / ACT (~706ns).  Everything
else is arranged to keep those two engines saturated:
  - h_obs pre-transposed on host -> no feature-net transposes on device.
  - W^T for attention via dma_start_transpose (sync queue + DMA engines).
  - feature net runs at startup; attention split blocks 0-5 / 6-7.
  - per-block exp/normalize emitted at lagged positions in the engine streams.
"""

import os
import sys
import types
import numpy as np

B, N_O, N_Q = 2, 512, 512
POS_DIM, LATENT, HEADS, HEAD_DIM, OUT_DIM = 3, 256, 8, 32, 128
HD = HEADS * HEAD_DIM
N_CORES = 8
NQ_C = N_Q * B // N_CORES  # 128 queries per core
QB = 16                     # queries per logits block
NBLK = NQ_C // QB           # 8 blocks per core

LAST_RESULT = None  # test.py reads exec_time_ns from here

N_A_TILES = int(os.environ.get("KERNEL_NA", "79"))  # relu tiles given to ACT
EXP_LAG_POS = int(os.environ.get("KERNEL_EXPLAG", "12"))   # tile idx in next blk
NORM_LAG_POS = int(os.environ.get("KERNEL_NORMLAG", "20"))


def _relu_pattern():
    """256-char D/A assignment, A spread evenly (N_A_TILES of them)."""
    pat = []
    acc = 0
    for t in range(256):
        acc += N_A_TILES
        if acc >= 256:
            acc -= 256
            pat.append("A")
        else:
            pat.append("D")
    return "".join(pat)


def _install_ntff_hook():
    if "antenv.axon_hooks" in sys.modules:
        return
    try:
        import trn_agent_boot.trn_boot as tb
        hook = tb._ntff_profile_via_ctypes("/opt/axon/libaxon_pjrt.so")
    except Exception:
        hook = None
    m = types.ModuleType("antenv.axon_hooks")
    m.get_axon_ntff_profile_hook = lambda: hook
    m.set_axon_ntff_profile_hook = lambda h: None
    sys.modules["antenv.axon_hooks"] = m


def build_program():
    import concourse.bass as bass
    import concourse.mybir as mybir
    import concourse.tile as tile
    from concourse import bacc
    from contextlib import ExitStack

    f32 = mybir.dt.float32
    bf16 = mybir.dt.bfloat16
    ALU = mybir.AluOpType
    AF = mybir.ActivationFunctionType

    nc = bacc.Bacc("TRN2", target_bir_lowering=False, debug=False)

    def din(name, shape):
        return nc.dram_tensor(name, shape, f32, kind="ExternalInput").ap()

    def dout(name, shape):
        return nc.dram_tensor(name, shape, f32, kind="ExternalOutput").ap()

    h_obs_T = din("h_obs_T", [LATENT, N_O])
    pos_obs_T = din("pos_obs_T", [POS_DIM, N_O])
    pos_q_T = din("pos_q_T", [POS_DIM, NQ_C])
    fw1 = din("fw1", [LATENT, LATENT])
    fb1 = din("fb1", [LATENT])
    fw2 = din("fw2", [LATENT, HD])
    fb2 = din("fb2", [HD])
    log_sigma = din("log_sigma", [HEADS])
    kw1 = din("kw1", [POS_DIM * 3, LATENT])
    kb1 = din("kb1", [LATENT])
    kw2 = din("kw2", [LATENT, HEADS])
    ow = din("ow", [HD, OUT_DIM])
    ob = din("ob", [OUT_DIM])
    vw = din("vw", [HD, OUT_DIM])
    vb = din("vb", [OUT_DIM])
    mean_o = dout("mean", [NQ_C, OUT_DIM])
    var_o = dout("var", [NQ_C, OUT_DIM])

    def ap(t, offset, dims):
        return bass.AP(tensor=t.tensor, offset=t.offset + offset, ap=list(dims))

    PAT = _relu_pattern()

    with tile.TileContext(nc) as tc:
        st = ExitStack()
        _keep = []

        def T(shape, name, dt=f32):
            t, free = tc.tile(shape, dt, name=name)
            _keep.append(free)
            return t

        # ---------------- persistent SBUF tiles ----------------
        ls_rep = T([128, 1], "ls_rep")        # log_sigma[h] at partition (q*8+h)
        s2inv_neg = T([128, 1], "s2inv_neg")  # -1/(exp(2 ls)+1e-6)
        s2rep4 = T([4, 128], "s2rep4")
        w_bd = [T([128, 128], f"w_bd{k}", bf16) for k in range(2)]
        kw2_2 = T([128, 2 * HEADS], "kw2_2")
        k3 = T([3, 3 * LATENT], "k3")         # kw1 blocks A|B|C side by side
        AC = T([3, LATENT], "AC")
        BC = T([3, LATENT], "BC")
        lhsT_df = T([3, NQ_C], "lhsT_df")
        pq4 = T([4, NQ_C], "pq4")
        pos_oT = T([3, N_O], "pos_oT")
        rhs_d = T([4, N_O], "rhs_d", bf16)
        sq3 = T([3, N_O], "sq3")
        ones3 = T([3, 1], "ones3")
        ones1 = T([1, 128], "ones1")
        tmp_po2 = T([1, N_O], "tmp_po2", bf16)
        dh_all = T([4, NBLK * 128], "dh_all", bf16)
        u_kt = [T([128, NQ_C], f"u_kt{k}") for k in range(2)]
        wT = [T([128, N_O], f"wT{k}", bf16) for k in range(2)]
        kb1_2 = T([128, 2], "kb1_2")
        fb1_2 = T([128, 2], "fb1_2")
        fb2_row = T([1, HD], "fb2_row")
        ob_row = T([1, OUT_DIM], "ob_row")
        vb_row = T([1, OUT_DIM], "vb_row")
        fw1_all = T([128, 2 * LATENT], "fw1_all")
        fw2_all = T([128, 2 * HD], "fw2_all")
        ow_all = T([128, 2 * OUT_DIM], "ow_all")
        vw_all = T([128, 2 * OUT_DIM], "vw_all")
        hT_all = T([128, 2 * N_O], "hT_all")
        hidT = [T([128, N_O], f"hidT{k}") for k in range(2)]
        v_sb = [T([128, HD], f"v{k}", bf16) for k in range(4)]
        v2_sb = [T([128, HD], f"v2{k}", bf16) for k in range(4)]
        WT_all = T([128, 4 * NQ_C * HEADS], "WT_all", bf16)  # [o, ot, i, (j,h)]
        sums_all = T([128, NBLK], "sums_all")
        recip_b = [T([128, 1], f"recip{i}") for i in range(NBLK)]
        hqT = [T([128, NQ_C], f"hqT{k}") for k in range(2)]
        sqT = [T([128, NQ_C], f"sqT{k}") for k in range(2)]
        varT = [T([128, NQ_C], f"varT{k}") for k in range(2)]
        mean_sb = T([NQ_C, OUT_DIM], "mean_sb")
        var_sb = T([NQ_C, OUT_DIM], "var_sb")

        # ---------------- DMA issue (spread across queues) ----------------
        # scalar queue: pos_q_T + fb1 (feeds psu -> u_kt quickly)
        nc.scalar.dma_start(out=lhsT_df[:], in_=pos_q_T[:])
        nc.scalar.dma_start(out=kb1_2[:], in_=ap(kb1, 0, [[1, 128], [128, 2]]))
        nc.scalar.dma_start(out=fb1_2[:], in_=ap(fb1, 0, [[1, 128], [128, 2]]))
        # sync queue (HWDGE): pos_obs_T, weights, then big loads
        nc.sync.dma_start(out=pos_oT[:], in_=pos_obs_T[:])
        nc.sync.dma_start(
            out=k3[:], in_=ap(kw1, 0, [[LATENT, 3], [3 * LATENT, 3], [1, LATENT]]))
        nc.sync.dma_start(
            out=kw2_2[:],
            in_=ap(kw2, 0, [[HEADS, 128], [128 * HEADS, 2], [1, HEADS]]))
        nc.sync.dma_start(
            out=hT_all[:],
            in_=ap(h_obs_T, 0, [[N_O, 128], [128 * N_O, 2], [1, N_O]]))
        nc.sync.dma_start(
            out=fw1_all[:],
            in_=ap(fw1, 0, [[LATENT, 128], [128 * LATENT, 2], [1, LATENT]]))
        nc.sync.dma_start(
            out=fw2_all[:],
            in_=ap(fw2, 0, [[HD, 128], [128 * HD, 2], [1, HD]]))
        nc.sync.dma_start(
            out=ow_all[:],
            in_=ap(ow, 0, [[OUT_DIM, 128], [128 * OUT_DIM, 2], [1, OUT_DIM]]))
        nc.sync.dma_start(
            out=vw_all[:],
            in_=ap(vw, 0, [[OUT_DIM, 128], [128 * OUT_DIM, 2], [1, OUT_DIM]]))
        # gpsimd queue (SWDGE, slow per descriptor): tiny transfers only
        nc.gpsimd.dma_start(out=ls_rep[:], in_=ap(log_sigma, 0, [[0, 16], [1, 8]]))
        nc.gpsimd.dma_start(out=pq4[0:3, :], in_=pos_q_T[:])
        nc.gpsimd.dma_start(out=fb2_row[:], in_=fb2[:])
        nc.gpsimd.dma_start(out=ob_row[:], in_=ob[:])
        nc.gpsimd.dma_start(out=vb_row[:], in_=vb[:])

        # ---------------- startup compute ----------------
        # s2 chain: exp on ACT, rest tiny DVE ops
        nc.scalar.activation(out=s2inv_neg[:], in_=ls_rep[:], func=AF.Exp, scale=2.0)
        nc.vector.memset(ones3[:], 1.0)
        nc.vector.memset(ones1[:], 1.0)
        nc.vector.tensor_scalar_add(out=s2inv_neg[:], in0=s2inv_neg[:], scalar1=1e-6)
        nc.vector.reciprocal(out=s2inv_neg[:], in_=s2inv_neg[:])
        nc.vector.tensor_add(AC[:], k3[:, 0:LATENT], k3[:, 2 * LATENT:3 * LATENT])
        nc.vector.tensor_sub(BC[:], k3[:, LATENT:2 * LATENT], k3[:, 2 * LATENT:])
        nc.scalar.mul(out=s2inv_neg[:], in_=s2inv_neg[:], mul=-1.0)
        nc.scalar.mul(out=rhs_d[0:3, :], in_=pos_oT[:], mul=-2.0)

        # remaining gpsimd DMAs (s2rep4 needs s2inv_neg computed above)
        nc.gpsimd.dma_start(out=pq4[3:4, :], in_=ones1[0:1, :])
        for r in range(4):
            nc.gpsimd.dma_start(out=s2rep4[r:r + 1, :], in_=s2inv_neg[:, 0:1])

        pp_feat = st.enter_context(tc.tile_pool(name="pp_feat", bufs=2, space="PSUM"))
        st0 = st.enter_context(ExitStack())
        pp_pre = st0.enter_context(tc.tile_pool(name="pp_pre", bufs=2, space="PSUM"))

        # u^T and w^T (kernel-MLP rank factors)
        for mt in range(2):
            psw = pp_pre.tile([128, N_O], f32, tag="pre", name="psw")
            nc.tensor.matmul(
                psw[:], lhsT=BC[:, 128 * mt:128 * (mt + 1)], rhs=pos_oT[:],
                start=True, stop=True)
            nc.scalar.activation(
                out=wT[mt][:], in_=psw[:], func=AF.Identity,
                bias=kb1_2[:, mt:mt + 1])
            psu = pp_pre.tile([128, NQ_C], f32, tag="pre", name="psu")
            nc.tensor.matmul(
                psu[:], lhsT=AC[:, 128 * mt:128 * (mt + 1)], rhs=lhsT_df[:],
                start=True, stop=True)
            nc.vector.tensor_copy(out=u_kt[mt][:], in_=psu[:])

        # block-diagonal kw2 (col-tiled): variant jj at cols [40jj, 40jj+8)
        for k in range(2):
            nc.vector.memset(w_bd[k][:], 0.0)
            for jj in range(4):
                nc.vector.tensor_copy(
                    out=w_bd[k][:, 40 * jj:40 * jj + 8],
                    in_=kw2_2[:, 8 * k:8 * (k + 1)])

        # ------- deferred chunks (emitted at block boundaries in the loop) ----
        def chunk_pre_dist():
            # |pos_o|^2 row of rhs_d + dist2 lhsT tiles (needed by dist2 MM of
            # block 0, which runs at the end of block 0's accumulation)
            nc.vector.tensor_mul(sq3[:], pos_oT[:], pos_oT[:])
            ps1 = pp_feat.tile([1, N_O], f32, tag="f", name="ps1")
            nc.tensor.matmul(ps1[:], lhsT=ones3[:], rhs=sq3[:], start=True, stop=True)
            nc.vector.tensor_copy(out=tmp_po2[:], in_=ps1[:])
            nc.gpsimd.dma_start(out=rhs_d[3:4, :], in_=tmp_po2[:])
            for i in range(NBLK):
                _o = dh_all[:, 128 * i:128 * (i + 1)]
                _p = pq4[:]
                _s = s2rep4[:]
                nc.vector.tensor_tensor(
                    out=ap(_o, 0, [_o.ap[0], [8, QB], [1, 8]]),
                    in0=ap(_p, QB * i, [_p.ap[0], [1, QB], [0, 8]]),
                    in1=ap(_s, 0, [_s.ap[0], [8, QB], [1, 8]]),
                    op=ALU.mult)

        def chunk_feat_hid():
            for mt in range(2):
                psh = pp_feat.tile([128, N_O], f32, tag="f", name="psh")
                for kt in range(2):
                    nc.tensor.matmul(
                        psh[:],
                        lhsT=fw1_all[:, 256 * kt + 128 * mt:256 * kt + 128 * (mt + 1)],
                        rhs=hT_all[:, 512 * kt:512 * (kt + 1)],
                        start=(kt == 0), stop=(kt == 1))
                nc.scalar.activation(
                    out=hidT[mt][:], in_=psh[:], func=AF.Relu,
                    bias=fb1_2[:, mt:mt + 1])

        def chunk_feat_v(ots):
            def f():
                for ot in ots:
                    psv = pp_feat.tile([128, HD], f32, tag="f", name="psv")
                    for mt in range(2):
                        nc.tensor.matmul(
                            psv[:], lhsT=hidT[mt][:, 128 * ot:128 * (ot + 1)],
                            rhs=fw2_all[:, 256 * mt:256 * (mt + 1)],
                            start=(mt == 0), stop=False)
                    nc.tensor.matmul(psv[:], lhsT=ones1[:], rhs=fb2_row[:],
                                     start=False, stop=True)
                    nc.scalar.copy(out=v_sb[ot][:], in_=psv[:])
                    nc.vector.tensor_mul(v2_sb[ot][:], v_sb[ot][:], v_sb[ot][:])
            return f

        # ---------------- block loop ----------------
        st0.close()
        spool = st.enter_context(tc.tile_pool(name="spool", bufs=28))
        wpool = st.enter_context(tc.tile_pool(name="wpool", bufs=3))
        pp_l = st.enter_context(tc.tile_pool(name="pp_l", bufs=3, space="PSUM"))
        pp_att = st.enter_context(tc.tile_pool(name="pp_att", bufs=1, space="PSUM"))

        pm_all = pp_att.tile([128, 2 * NQ_C], f32, tag="pm", name="pm_all")
        pe_all = pp_att.tile([128, 2 * NQ_C], f32, tag="pe", name="pe_all")
        pm = [pm_all[:, NQ_C * k:NQ_C * (k + 1)] for k in range(2)]
        pe = [pe_all[:, NQ_C * k:NQ_C * (k + 1)] for k in range(2)]

        W_ts = [None] * NBLK
        lps_all = [None] * NBLK

        def emit_post(i):
            """exp + recip + normalize + dma-transpose for block i."""
            lps = lps_all[i]
            W_t = wpool.tile([128, N_O], bf16, tag="W", name="W_t")
            W_ts[i] = W_t
            nc.scalar.activation(
                out=W_t[:], in_=lps[:], func=AF.Exp,
                accum_out=sums_all[:, i:i + 1])
            nc.vector.reciprocal(out=recip_b[i][:], in_=sums_all[:, i:i + 1])
            nc.vector.tensor_scalar_mul(
                out=W_t[:], in0=W_t[:], scalar1=recip_b[i][:, 0:1])
            _w = WT_all[:]
            out3d = bass.AP(
                tensor=_w.tensor, offset=_w.offset + 128 * i,
                ap=[_w.ap[0], [NQ_C * HEADS, 4], [1, 128]])
            nc.sync.dma_start_transpose(out=out3d, in_=W_t[:])

        def emit_block(i):
            lps = pp_l.tile([128, N_O], f32, tag="logits", name="lps")
            lps_all[i] = lps
            tidx = 0
            for kt in range(2):
                for wave in range(4):
                    stiles = []
                    for g in range(4):
                        j = 4 * g + wave
                        qg = QB * i + j
                        s_t = spool.tile([128, N_O], bf16, tag="s", name="s_t")
                        ucol = u_kt[kt][:, qg:qg + 1]
                        eng = PAT[32 * i + tidx]
                        if eng == "D":
                            nc.vector.tensor_scalar(
                                out=s_t[:], in0=wT[kt][:], scalar1=ucol,
                                scalar2=0.0, op0=ALU.add, op1=ALU.max)
                        else:
                            nc.scalar.activation(
                                out=s_t[:], in_=wT[kt][:], func=AF.Relu,
                                bias=ucol)
                        stiles.append((g, s_t))
                        tidx += 1
                        if i >= 1 and tidx == EXP_LAG_POS:
                            emit_post(i - 1)
                    for g, s_t in stiles:
                        nc.tensor.matmul(
                            lps[32 * g:32 * g + 32, :],
                            lhsT=w_bd[kt][:, 32 * wave:32 * (wave + 1)],
                            rhs=s_t[:],
                            start=(kt == 0 and wave == 0), stop=False,
                            tile_position=(0, 32 * g), skip_group_check=True)
            nc.tensor.matmul(
                lps[:], lhsT=dh_all[:, 128 * i:128 * (i + 1)], rhs=rhs_d[:],
                start=False, stop=True, skip_group_check=True)

        def emit_attention(i0, i1):
            """accumulate pm/pe for blocks [i0, i1); h-inner rotates strips."""
            nb = i1 - i0
            c0 = QB * i0
            for dst, vs in ((pm, v_sb), (pe, v2_sb)):
                for ot in range(4):
                    for h in range(HEADS):
                        k = h // 4
                        r0 = 32 * (h % 4)
                        rhs = ap(WT_all[:], 1024 * ot + 128 * i0 + h,
                                 [WT_all[:].ap[0], [128, nb], [8, QB]])
                        nc.tensor.matmul(
                            dst[k][r0:r0 + 32, c0:c0 + QB * nb],
                            lhsT=vs[ot][:, 32 * h:32 * (h + 1)], rhs=rhs,
                            start=(ot == 0), stop=(ot == 3), tile_position=(0, r0))

        chunks = {0: [chunk_pre_dist, chunk_feat_hid],
                  1: [chunk_feat_v([0, 1])],
                  2: [chunk_feat_v([2, 3])]}
        for i in range(NBLK):
            emit_block(i)
            for c in chunks.get(i, []):
                c()
            if i == 6:
                emit_attention(0, 6)
        emit_post(NBLK - 1)
        emit_attention(6, 8)

        # ---------------- tail: mean/var ----------------
        for k in range(2):
            nc.vector.tensor_copy(out=hqT[k][:], in_=pm[k][:])
            nc.vector.tensor_mul(sqT[k][:], hqT[k][:], hqT[k][:])
            nc.vector.tensor_sub(varT[k][:], pe[k][:], sqT[k][:])

        pso = pp_feat.tile([NQ_C, OUT_DIM], f32, tag="f", name="pso")
        for k in range(2):
            nc.tensor.matmul(pso[:], lhsT=hqT[k][:], rhs=ow_all[:, 128 * k:128 * (k + 1)],
                             start=(k == 0), stop=False)
        nc.tensor.matmul(pso[:], lhsT=ones1[:], rhs=ob_row[:], start=False, stop=True)
        nc.vector.tensor_copy(out=mean_sb[:], in_=pso[:])
        nc.sync.dma_start(out=mean_o[:], in_=mean_sb[:])

        psv2 = pp_feat.tile([NQ_C, OUT_DIM], f32, tag="f", name="psv2")
        for k in range(2):
            nc.tensor.matmul(psv2[:], lhsT=varT[k][:], rhs=vw_all[:, 128 * k:128 * (k + 1)],
                             start=(k == 0), stop=False)
        nc.tensor.matmul(psv2[:], lhsT=ones1[:], rhs=vb_row[:], start=False, stop=True)
        # softplus(x) = ln(1 + exp(x))
        nc.scalar.activation(out=var_sb[:], in_=psv2[:], func=AF.Exp)
        nc.vector.tensor_scalar_add(out=var_sb[:], in0=var_sb[:], scalar1=1.0)
        nc.scalar.activation(out=var_sb[:], in_=var_sb[:], func=AF.Ln)
        nc.sync.dma_start(out=var_o[:], in_=var_sb[:])

        st.close()
        for f in reversed(_keep):
            f()

    nc.compile()
    return nc


_NC = None


def _get_nc():
    global _NC
    if _NC is None:
        _NC = build_program()
    return _NC


def shard_inputs(inputs):
    """Build per-core input maps from full inputs."""
    g = {k: np.ascontiguousarray(np.asarray(v, dtype=np.float32))
         for k, v in inputs.items()}
    hT = [np.ascontiguousarray(g["h_obs"][b].T) for b in range(B)]
    poT = [np.ascontiguousarray(g["pos_obs"][b].T) for b in range(B)]
    maps = []
    for c in range(N_CORES):
        b, qi = c // 4, c % 4
        maps.append({
            "h_obs_T": hT[b],
            "pos_obs_T": poT[b],
            "pos_q_T": np.ascontiguousarray(
                g["pos_query"][b, 128 * qi:128 * (qi + 1)].T),
            "fw1": g["fw1"], "fb1": g["fb1"], "fw2": g["fw2"], "fb2": g["fb2"],
            "log_sigma": g["log_sigma"],
            "kw1": g["kw1"], "kb1": g["kb1"], "kw2": g["kw2"],
            "ow": g["ow"], "ob": g["ob"], "vw": g["vw"], "vb": g["vb"],
        })
    return maps


def kernel(**inputs):
    global LAST_RESULT
    _install_ntff_hook()
    from concourse.bass_utils import run_bass_kernel_spmd

    nc = _get_nc()
    maps = shard_inputs(inputs)
    trace = bool(int(os.environ.get("KERNEL_TRACE", "0")))
    res = run_bass_kernel_spmd(nc, maps, list(range(N_CORES)), trace=trace)
    LAST_RESULT = res
    mean = np.zeros((B, N_Q, OUT_DIM), np.float32)
    var = np.zeros((B, N_Q, OUT_DIM), np.float32)
    for c in range(N_CORES):
        b, qi = c // 4, c % 4
        mean[b, 128 * qi:128 * (qi + 1)] = res.results[c]["mean"]
        var[b, 128 * qi:128 * (qi + 1)] = res.results[c]["var"]
    return (mean, var)
